# revision 1
# baseline (speedup 1.0000x reference)
"""Word-encoder masked-attention pooling (segment softmax-reduce) on 8 trn2 cores.

Strategy (sharding_hint): shard the n_words dimension across the 8 cores
(750 words each).  Spans are contiguous and sorted, so each 128-word tile
only touches a small contiguous slice of hidden_states.  The host gathers,
per word-tile, the KB*128 hidden rows covering that tile's spans plus a
0/1 span mask (transposed, [subword, word]); the device then computes

    s      = H_rows . w_attn                  (DVE fused mul-reduce)
    E      = exp(s)                           (ACT)
    num    = (mask^T * E)^T @ H_rows          (PE f32r, accumulated over KB)
    den    = mask^T^T @ E                     (PE f32r)
    out    = num * (1/den)                    (DVE recip + ACT copy-scale)

which equals softmax(masked scores) @ hidden_states exactly (the constant
b_attn cancels in the softmax).  No cross-core communication.
"""

import ml_dtypes
import numpy as np
from contextlib import ExitStack

import concourse.bass as bass
import concourse.bacc as bacc
import concourse.mybir as mybir
import concourse.tile as tile
from concourse.bass_utils import run_bass_kernel_spmd

NCORES = 8
P = 128
HID = 1024

LAST_RESULT = None  # BassKernelResults of the most recent run (for profiling)

_prog_cache = {}


def _build_program(MT, KBs):
    """One SPMD program for all cores. MT word-tiles of 128 words; word-tile m
    consumes KBs[m] gathered k-blocks of 128 subword rows."""
    T = sum(KBs)
    f32 = mybir.dt.float32
    f32r = mybir.dt.float32r
    bf16 = mybir.dt.bfloat16
    nc = bacc.Bacc(
        "TRN2", target_bir_lowering=False, debug=False, num_devices=NCORES
    )
    hg = nc.declare_dram_parameter("hg", [T * P, HID], f32, isOutput=False)
    mk = nc.declare_dram_parameter("mk", [T * P, P], mybir.dt.bfloat16, isOutput=False)
    wb = nc.declare_dram_parameter("wb", [P, HID], f32, isOutput=False)
    out = nc.declare_dram_parameter("out", [MT * P, HID], f32, isOutput=True)

    with tile.TileContext(nc) as tc, ExitStack() as ctx:
        T = sum(KBs)
        wpool = ctx.enter_context(tc.tile_pool(name="w", bufs=1))
        hpool = ctx.enter_context(tc.tile_pool(name="h", bufs=T))
        mpool = ctx.enter_context(tc.tile_pool(name="m", bufs=T))
        mepool = ctx.enter_context(tc.tile_pool(name="me", bufs=6))
        prodpool = ctx.enter_context(tc.tile_pool(name="prod", bufs=3))
        spool = ctx.enter_context(tc.tile_pool(name="s", bufs=16))
        opool = ctx.enter_context(tc.tile_pool(name="o", bufs=3))
        pnpool = ctx.enter_context(tc.tile_pool(name="pn", bufs=3, space="PSUM"))
        pdpool = ctx.enter_context(tc.tile_pool(name="pd", bufs=2, space="PSUM"))

        wt = wpool.tile([P, HID], f32)
        nc.sync.dma_start(wt[:], wb[:, :])
        ones8 = wpool.tile([P, 8], f32r)
        nc.vector.memset(ones8[:].bitcast(f32), 1.0)

        t = 0
        for m in range(MT):
            pn = pnpool.tile([P, HID], f32)
            pd = pdpool.tile([P, 8], f32)
            nkb = KBs[m]
            for kb in range(nkb):
                h = hpool.tile([P, HID], f32r)
                heng = nc.sync if t % 2 == 0 else nc.gpsimd
                heng.dma_start(h[:], hg[t * P : (t + 1) * P, :].bitcast(f32r))
                mkt = mpool.tile([P, P], bf16)
                nc.gpsimd.dma_start(mkt[:], mk[t * P : (t + 1) * P, :])

                # s = sum_f h*w  (one fused DVE pass); E = exp(s) on ACT
                prod = prodpool.tile([P, HID], f32)
                s = spool.tile([P, 1], f32)
                nc.vector.scalar_tensor_tensor(
                    out=prod[:],
                    in0=h[:].bitcast(f32),
                    scalar=1.0,
                    in1=wt[:],
                    op0=mybir.AluOpType.mult,
                    op1=mybir.AluOpType.mult,
                    accum_out=s[:],
                )
                e = spool.tile([P, 1], f32)
                nc.scalar.activation(
                    e[:], s[:], mybir.ActivationFunctionType.Exp
                )

                mke = mepool.tile([P, P], f32r)
                nc.scalar.activation(
                    mke[:], mkt[:], mybir.ActivationFunctionType.Copy, scale=e[:]
                )

                first, last = kb == 0, kb == nkb - 1
                nc.tensor.matmul(pd[:], mke[:], ones8[:], start=first, stop=last)
                for half in range(2):
                    cs = slice(half * 512, (half + 1) * 512)
                    nc.tensor.matmul(
                        pn[:, cs], mke[:], h[:, cs], start=first, stop=last
                    )
                t += 1

            r = spool.tile([P, 1], f32)
            nc.vector.reciprocal(r[:], pd[:, 0:1])
            o = opool.tile([P, HID], f32)
            nc.scalar.activation(
                o[:], pn[:], mybir.ActivationFunctionType.Copy, scale=r[:]
            )
            nc.sync.dma_start(out[m * P : (m + 1) * P, :], o[:])

    nc.compile()
    return nc


def kernel(hidden_states, word_starts, word_ends, w_attn, b_attn):
    global LAST_RESULT
    H = np.ascontiguousarray(np.asarray(hidden_states, dtype=np.float32))
    ws = np.asarray(word_starts).astype(np.int64)
    we = np.asarray(word_ends).astype(np.int64)
    wv = np.asarray(w_attn, dtype=np.float32).reshape(-1)
    ns, hid = H.shape
    nw = ws.shape[0]
    assert hid == HID
    Wpc = (nw + NCORES - 1) // NCORES  # words per core
    MT = (Wpc + P - 1) // P  # word-tiles per core

    def tile_bounds(c, m):
        lo = c * Wpc + m * P
        hi = min(lo + P, (c + 1) * Wpc, nw)
        return lo, hi

    # k-blocks per word-tile position: max over cores of the tile's span range
    KBs = []
    for m in range(MT):
        kb = 1
        for c in range(NCORES):
            lo, hi = tile_bounds(c, m)
            if lo >= hi:
                continue
            R = int(we[lo:hi].max() - ws[lo] + 1)
            kb = max(kb, (max(R, 1) + P - 1) // P)
        KBs.append(kb)
    T = sum(KBs)

    wb = np.ascontiguousarray(np.broadcast_to(wv[None, :], (P, HID)))
    in_maps = []
    for c in range(NCORES):
        Hg = np.zeros((T * P, HID), np.float32)
        Mk = np.zeros((T * P, P), ml_dtypes.bfloat16)
        t0 = 0
        for m in range(MT):
            lo, hi = tile_bounds(c, m)
            kb = KBs[m]
            if lo < hi:
                kstart = int(ws[lo])
                rows = min(kb * P, ns - kstart)
                Hg[t0 * P : t0 * P + rows] = H[kstart : kstart + rows]
                a = (ws[lo:hi] - kstart).astype(np.int64)
                b = (we[lo:hi] - kstart).astype(np.int64)
                j = np.arange(kb * P, dtype=np.int64)[:, None]
                Mk[t0 * P : (t0 + kb) * P, : hi - lo] = (
                    (j >= a[None, :]) & (j <= b[None, :])
                ).astype(ml_dtypes.bfloat16)
                if hi - lo < P:
                    # padding word slots: all weight on row 0 (finite, discarded)
                    Mk[t0 * P, hi - lo :] = 1.0
            else:
                Mk[t0 * P, :] = 1.0
            t0 += kb
        in_maps.append({"hg": Hg, "mk": Mk, "wb": wb})

    key = (MT, tuple(KBs))
    nc = _prog_cache.get(key)
    if nc is None:
        nc = _build_program(MT, KBs)
        _prog_cache[key] = nc

    res = run_bass_kernel_spmd(nc, in_maps, list(range(NCORES)))
    LAST_RESULT = res
    full = np.concatenate(
        [res.results[c]["out"][:Wpc] for c in range(NCORES)], axis=0
    )[:nw]
    return np.ascontiguousarray(full, dtype=np.float32)



# revision 5
# speedup vs baseline: 1.3848x; 1.3848x over previous
"""Word-encoder masked-attention pooling (segment softmax-reduce) on 8 trn2 cores.

Sharding: n_words split across 8 cores (750 words each).  Spans are sorted
and contiguous, so each 128-word tile touches a small contiguous band of
hidden_states rows.  Host gathers, per word-tile, KB*128 hidden rows as
bf16 (plus a trailing ones column) and a 0/1 span mask in fp8.

Device, per 128-row block t (all bf16 datapath, f32 PSUM):

    s'     = sum_f h*w + 30          (one DVE fused mul-reduce; wb[1024]=30)
    mke    = Exp(mask * s' - 30)     (one ACT op: exp+mask+broadcast fused;
                                      mask=0 -> e^-30 ~ 1e-13 ~ 0)
    pn    += mke^T @ h               (PE bf16, two 512-wide matmuls)
    pd    += mke^T @ ones_col        (PE bf16, 1-wide matmul)

then per word tile: r = 1/pd (DVE), out = pn * r -> bf16 (gpsimd reads
PSUM), DMA out.  Host upcasts to f32.  b_attn==const cancels in softmax.
"""

import ml_dtypes
import numpy as np
from contextlib import ExitStack

import concourse.bass as bass
import concourse.bacc as bacc
import concourse.mybir as mybir
import concourse.tile as tile
from concourse.bass_utils import run_bass_kernel_spmd

NCORES = 8
P = 128
HID = 1024
HB = HID + 1  # block width: 1024 hidden cols + ones col

LAST_RESULT = None  # BassKernelResults of the most recent run (for profiling)

_prog_cache = {}


def _build_program(MT, KBs):
    """One SPMD program for all cores. MT word-tiles of 128 words; word-tile m
    consumes KBs[m] gathered k-blocks of 128 subword rows."""
    T = sum(KBs)
    f32 = mybir.dt.float32
    bf16 = mybir.dt.bfloat16
    fp8 = mybir.dt.float8e4
    EXP = mybir.ActivationFunctionType.Exp
    nc = bacc.Bacc(
        "TRN2", target_bir_lowering=False, debug=False, num_devices=NCORES
    )
    hg = nc.declare_dram_parameter("hg", [P, T * HB], bf16, isOutput=False)
    mk = nc.declare_dram_parameter("mk", [P, T * P], fp8, isOutput=False)
    wb = nc.declare_dram_parameter("wb", [P, HB], bf16, isOutput=False)
    out = nc.declare_dram_parameter("out", [P, MT * HID], bf16, isOutput=True)

    with tile.TileContext(nc) as tc, ExitStack() as ctx:
        wpool = ctx.enter_context(tc.tile_pool(name="w", bufs=1))
        hpool = ctx.enter_context(tc.tile_pool(name="h", bufs=MT))
        mpool = ctx.enter_context(tc.tile_pool(name="m", bufs=1))
        mepool = ctx.enter_context(tc.tile_pool(name="me", bufs=4))
        prodpool = ctx.enter_context(tc.tile_pool(name="prod", bufs=2))
        spool = ctx.enter_context(tc.tile_pool(name="s", bufs=4))
        rpool = ctx.enter_context(tc.tile_pool(name="r", bufs=2))
        opool = ctx.enter_context(tc.tile_pool(name="o", bufs=3))
        pnpool = ctx.enter_context(tc.tile_pool(name="pn", bufs=3, space="PSUM"))
        pdpool = ctx.enter_context(tc.tile_pool(name="pd", bufs=2, space="PSUM"))

        # weights + masks on the gpsimd ring; h chunks on the sync ring
        # (separate queues -> transfers overlap)
        wt = wpool.tile([P, HB], bf16)
        nc.gpsimd.dma_start(wt[:], wb[:, :])
        nbias = wpool.tile([P, 1], f32)
        nc.vector.memset(nbias[:], -30.0)
        mkt = mpool.tile([P, T * P], fp8)
        nc.gpsimd.dma_start(mkt[:], mk[:, :])

        # one h-chunk DMA per word tile (KBs[m] blocks each)
        htiles = []
        t0 = 0
        for m in range(MT):
            ht = hpool.tile([P, KBs[m] * HB], bf16)
            nc.sync.dma_start(ht[:], hg[:, t0 * HB : (t0 + KBs[m]) * HB])
            htiles.append(ht)
            t0 += KBs[m]

        def emit_tail(m, pn, pd):
            # per-word-tile epilogue: r = 1/den, out = pn*r -> bf16, DMA out.
            # Emitted one tile late (software pipelining) so the ACT out-scale
            # does not sit in front of tile m+1's mke ops in the ACT stream.
            r = rpool.tile([P, 1], f32)
            nc.vector.reciprocal(r[:], pd[:, 0:1])
            o = opool.tile([P, HID], bf16)
            nc.scalar.activation(
                o[:], pn[:], mybir.ActivationFunctionType.Copy, scale=r[:]
            )
            nc.gpsimd.dma_start(out[:, m * HID : (m + 1) * HID], o[:])

        t = 0
        pending = None  # (m, pn, pd) awaiting tail emission
        for m in range(MT):
            pn = pnpool.tile([P, HID], f32)
            pd = pdpool.tile([P, 8], f32)
            nkb = KBs[m]
            ht = htiles[m]
            for kb in range(nkb):
                ones = ht[:, kb * HB + HID : kb * HB + HB]
                prod = prodpool.tile([P, HB], bf16)
                s = spool.tile([P, 1], f32)
                nc.vector.scalar_tensor_tensor(
                    out=prod[:],
                    in0=ht[:, kb * HB : (kb + 1) * HB],
                    scalar=1.0,
                    in1=wt[:],
                    op0=mybir.AluOpType.mult,
                    op1=mybir.AluOpType.mult,
                    accum_out=s[:],
                )
                mke = mepool.tile([P, P], bf16)
                nc.scalar.activation(
                    mke[:], mkt[:, t * P : (t + 1) * P], EXP,
                    bias=nbias[:], scale=s[:],
                )

                first, last = kb == 0, kb == nkb - 1
                nc.tensor.matmul(
                    pd[:, 0:1], mke[:], ones, start=first, stop=last
                )
                for half in range(2):
                    cs = slice(kb * HB + half * 512, kb * HB + (half + 1) * 512)
                    nc.tensor.matmul(
                        pn[:, half * 512 : (half + 1) * 512],
                        mke[:],
                        ht[:, cs],
                        start=first,
                        stop=last,
                    )
                t += 1

            if pending is not None:
                emit_tail(*pending)
            pending = (m, pn, pd)
        emit_tail(*pending)

    nc.compile()
    return nc


def kernel(hidden_states, word_starts, word_ends, w_attn, b_attn):
    global LAST_RESULT
    H = np.asarray(hidden_states, dtype=np.float32)
    ws = np.asarray(word_starts).astype(np.int64)
    we = np.asarray(word_ends).astype(np.int64)
    wv = np.asarray(w_attn, dtype=np.float32).reshape(-1)
    ns, hid = H.shape
    nw = ws.shape[0]
    assert hid == HID
    Hb = H.astype(ml_dtypes.bfloat16)
    Wpc = (nw + NCORES - 1) // NCORES  # words per core
    MT = (Wpc + P - 1) // P  # word-tiles per core

    def tile_bounds(c, m):
        lo = c * Wpc + m * P
        hi = min(lo + P, (c + 1) * Wpc, nw)
        return lo, hi

    # k-blocks per word-tile position: max over cores of the tile's span range
    KBs = []
    for m in range(MT):
        kb = 1
        for c in range(NCORES):
            lo, hi = tile_bounds(c, m)
            if lo >= hi:
                continue
            R = int(we[lo:hi].max() - ws[lo] + 1)
            kb = max(kb, (max(R, 1) + P - 1) // P)
        KBs.append(kb)
    T = sum(KBs)

    wb = np.zeros((P, HB), np.float32)
    wb[:, :HID] = wv[None, :]
    wb[:, HID] = 30.0
    wb = wb.astype(ml_dtypes.bfloat16)

    pos = np.arange(P, dtype=np.int64)
    in_maps = []
    for c in range(NCORES):
        Hg = np.zeros((P, T * HB), ml_dtypes.bfloat16)
        Mk = np.zeros((P, T * P), ml_dtypes.float8_e4m3)
        t0 = 0
        for m in range(MT):
            lo, hi = tile_bounds(c, m)
            kb = KBs[m]
            if lo < hi:
                kstart = int(ws[lo])
                a = ws[lo:hi] - kstart  # [nw_t] relative starts
                b = we[lo:hi] - kstart
                for k in range(kb):
                    j = kstart + k * P + pos  # global rows of this block
                    valid = j < ns
                    blk = Hg[:, (t0 + k) * HB : (t0 + k + 1) * HB]
                    blk[valid, :HID] = Hb[j[valid]]
                    blk[valid, HID] = 1.0
                    jr = (k * P + pos)[:, None]  # rows relative to kstart
                    msk = (jr >= a[None, :]) & (jr <= b[None, :]) & valid[:, None]
                    Mk[:, (t0 + k) * P : (t0 + k) * P + (hi - lo)] = msk.astype(
                        ml_dtypes.float8_e4m3
                    )
            t0 += kb
        in_maps.append({"hg": Hg, "mk": Mk, "wb": wb})

    key = (MT, tuple(KBs))
    nc = _prog_cache.get(key)
    if nc is None:
        nc = _build_program(MT, KBs)
        _prog_cache[key] = nc

    res = run_bass_kernel_spmd(nc, in_maps, list(range(NCORES)))
    LAST_RESULT = res
    parts = []
    for c in range(NCORES):
        o = np.asarray(res.results[c]["out"])  # [P, MT*HID] bf16
        o = o.reshape(P, MT, HID).transpose(1, 0, 2).reshape(MT * P, HID)
        parts.append(o[:Wpc])
    full = np.concatenate(parts, axis=0)[:nw]
    return np.ascontiguousarray(full.astype(np.float32))
